# revision 11
# baseline (speedup 1.0000x reference)
"""Trainium2 Bass kernel for nn_Attention_69544110457499 (sparse_attention).

Computes, per sample n and head h (no softmax, seq=1):
    k_cache[n, t] = k[n];  v_cache[n, t] = v[n]      (t = 777 % 4096)
    out[n, h]    = (q[n,h] @ K[n,:,h,:].T) @ V[n,:,h,:]

Key ideas:
  * Data-parallel over the sample axis S=64 -> 8 samples per NeuronCore,
    fully local, zero collectives.
  * Associativity: (q @ K^T) @ V == q @ (K^T @ V). K^T V contracts over the
    cache-row axis b, which is the *natural* partition layout of both caches
    ([b, h*d] tiles straight from DRAM) -- no transposes of the 805 MB of
    cache data, and the kernel is purely HBM-bandwidth bound.
  * K and V are interleaved host-side into one kv_cache input (one DMA per
    SBUF tile), and the row-t cache write is applied during that repack, so
    the device graph has no patch traffic at all.  Only `out` is returned by
    the reference, so the updated cache never needs to reach DRAM.
  * This walrus only allows ONE sync-wait per instruction.  The structure
    keeps every instruction at <=1 wait: eight cache tiles per sample cycle
    through eight pool slots so each slot always reuses the same HWDGE DMA
    lane (same-lane FIFO ordering needs no wait), tiny "toucher" matmuls
    absorb fresh-tile DMA waits before the real accumulation matmuls (which
    carry a PSUM-slot PE self-wait), and q/out DMAs ride the separate SWDGE
    lanes.
"""

import os
import sys

sys.path.insert(0, "/opt/trn_rl_repo")

from contextlib import ExitStack

import numpy as np

import concourse.bass as bass
import concourse.mybir as mybir
import concourse.tile as tile
from concourse import bacc
from concourse.bass_utils import run_bass_kernel_spmd

N_CORES = 8
S, SEQ, H, D = 64, 1, 12, 64
BLOCK = 2048
WINDOW = 4096
NS = S // N_CORES  # samples per core
HD = H * D  # 768
P = 128  # partitions / chunk rows
CHUNKS = BLOCK // P  # 16
NQTR = 8  # cache slices per sample (DMA granules)
CPQ = CHUNKS // NQTR  # chunks per slice
QROWS = CPQ * P  # cache rows per slice
QFREE = CPQ * HD  # per-section free dim of one slice tile
NPAIR = H // 2  # head pairs

F32 = mybir.dt.float32

# Filled by kernel(); test.py reads it.
LAST_RESULTS = None


def _build_nc() -> bass.Bass:
    """Build the per-core Bass graph (t handled host-side)."""
    nc = bacc.Bacc()

    q_ext = nc.declare_dram_parameter("q", [NS, SEQ, H, D], F32, isOutput=False)
    # caches interleaved per slice: kv_cache[n, qtr, 0]=k_cache rows,
    # kv_cache[n, qtr, 1]=v_cache rows (row t already patched host-side)
    kvc_ext = nc.declare_dram_parameter(
        "kv_cache", [NS, NQTR, 2, QROWS, H, D], F32, isOutput=False
    )
    out_ext = nc.declare_dram_parameter("out", [NS, SEQ, H, D], F32, isOutput=True)

    with tile.TileContext(nc) as tc, ExitStack() as ctx:
        cache_pool = ctx.enter_context(tc.tile_pool(name="cache", bufs=NQTR))
        ktv_pool = ctx.enter_context(tc.tile_pool(name="ktv", bufs=12))
        small_pool = ctx.enter_context(tc.tile_pool(name="small", bufs=1))
        outsb_pool = ctx.enter_context(tc.tile_pool(name="outsb", bufs=NS))
        acc_pool = ctx.enter_context(tc.tile_pool(name="acc", bufs=6, space="PSUM"))
        outp_pool = ctx.enter_context(tc.tile_pool(name="outp", bufs=2, space="PSUM"))

        # ---- q preparation (once) -------------------------------------
        # qsb: [96, 64] = q laid out (n h) x d, one contiguous DMA (SWDGE).
        qsb = small_pool.tile([NS * H, D], F32)
        nc.gpsimd.dma_start(
            out=qsb[:, :], in_=q_ext[:].rearrange("n s h d -> (n s h) d")
        )

        # qT: [64, 96] = d x (n h), via six 32x32 DVE block transposes.
        qT = small_pool.tile([D, NS * H], F32)
        for bi in range((NS * H) // 32):
            for bj in range(D // 32):
                nc.vector.transpose(
                    qT[32 * bj : 32 * (bj + 1), 32 * bi : 32 * (bi + 1)],
                    qsb[32 * bi : 32 * (bi + 1), 32 * bj : 32 * (bj + 1)],
                )

        # qx: zero-padded block-diagonal stationary for stage 2.
        # For (n, hp): columns [base, base+12); col 2hp rows 0:64 = q[n,2hp,:],
        # col 2hp+1 rows 64:128 = q[n,2hp+1,:]; everything else zero.
        qx = small_pool.tile([P, NS * NPAIR * H], F32)
        nc.vector.memset(qx[:, :], 0.0)
        for n in range(NS):
            for hp in range(NPAIR):
                base = (n * NPAIR + hp) * H
                nc.vector.tensor_copy(
                    qx[0:64, base + 2 * hp : base + 2 * hp + 1],
                    qT[0:64, n * H + 2 * hp : n * H + 2 * hp + 1],
                )
                nc.vector.tensor_copy(
                    qx[64:128, base + 2 * hp + 1 : base + 2 * hp + 2],
                    qT[0:64, n * H + 2 * hp + 1 : n * H + 2 * hp + 2],
                )

        # ---- main loop over samples -----------------------------------
        for n in range(NS):
            # Load the slice tiles of this sample's K+V cache segments.
            # Tile layout: [:, 0:QFREE] = K chunks, [:, QFREE:2*QFREE] = V.
            qtiles = []
            for qtr in range(NQTR):
                kv = cache_pool.tile(
                    [P, 2 * QFREE], F32, tag="kv", name=f"kv_{n}_{qtr}"
                )
                nc.sync.dma_start(
                    out=kv[:, :].rearrange("p (s c f) -> p s c f", s=2, c=CPQ),
                    in_=kvc_ext[:][n, qtr].rearrange("s (c p) h d -> p s c (h d)", p=P),
                )
                qtiles.append(kv)

            outp = outp_pool.tile([H, D], F32, tag="outp", name=f"outp_{n}")

            # One tiny matmul per fresh tile so the PE observes each tile's
            # DMA semaphore here; the real accumulation matmuls then carry
            # only their PSUM-slot PE self-wait (walrus allows one wait per
            # Matmult). Scribbles on outp[0,0], which stage 2 overwrites
            # (start=True clears the bank).
            for qtr in range(NQTR):
                nc.tensor.matmul(
                    outp[0:1, 0:1],
                    qtiles[qtr][0:1, 0:1],
                    qtiles[qtr][0:1, 0:1],
                    start=True,
                    stop=True,
                )

            # Two head-groups of 3 pairs each so stage-2 PSUM drains of one
            # group overlap stage-1 matmuls of the other (keeps PE warm and
            # fits 6+2 PSUM banks).
            for g in range(2):
                accs = [
                    acc_pool.tile([P, P], F32, tag="acc", name=f"acc_{n}_{g}_{j}")
                    for j in range(3)
                ]
                for qtr in range(NQTR):
                    kv = qtiles[qtr]
                    for c in range(CPQ):
                        cidx = qtr * CPQ + c
                        for i, hp in enumerate(range(3 * g, 3 * g + 3)):
                            koff = c * HD + hp * P
                            voff = QFREE + c * HD + hp * P
                            nc.tensor.matmul(
                                accs[i][:, :],
                                kv[:, koff : koff + P],
                                kv[:, voff : voff + P],
                                start=(cidx == 0),
                                stop=(cidx == CHUNKS - 1),
                            )
                # Stage 2: extract per-head diag blocks of K^T V, then the
                # tiny block-diagonal matmul q @ (K^T V) accumulating into
                # outp[12, 64].
                for i, hp in enumerate(range(3 * g, 3 * g + 3)):
                    ktv = ktv_pool.tile([P, D], F32, tag="ktv", name=f"ktv_{n}_{hp}")
                    nc.vector.tensor_copy(ktv[0:64, :], accs[i][0:64, 0:64])
                    nc.vector.tensor_copy(ktv[64:128, :], accs[i][64:128, 64:128])
                    base = (n * NPAIR + hp) * H
                    nc.tensor.matmul(
                        outp[:, :],
                        qx[:, base : base + H],
                        ktv[:, :],
                        start=(hp == 0),
                        stop=(hp == NPAIR - 1),
                    )

            osb = outsb_pool.tile([H, D], F32, tag="osb", name=f"osb_{n}")
            nc.vector.tensor_copy(osb[:, :], outp[:, :])
            nc.gpsimd.dma_start(
                out=out_ext[:][n].rearrange("s h d -> (s h) d"), in_=osb[:, :]
            )

    nc.compile()
    return nc


_NC_CACHE: dict[int, bass.Bass] = {}


def _get_nc() -> bass.Bass:
    if 0 not in _NC_CACHE:
        _NC_CACHE[0] = _build_nc()
    return _NC_CACHE[0]


def make_core_inputs(t_start, q, k, v, k_cache, v_cache, core: int):
    """Host-side shard + interleave (+ row-t cache write) for one core."""
    rows = slice(core * NS, (core + 1) * NS)

    kv = np.empty((NS, NQTR, 2, QROWS, H, D), dtype=np.float32)
    kv[:, :, 0] = k_cache[rows].reshape(NS, NQTR, QROWS, H, D)
    kv[:, :, 1] = v_cache[rows].reshape(NS, NQTR, QROWS, H, D)
    # the KV-cache write at row t (seq=1)
    qtr_t, row_t = divmod(t_start, QROWS)
    kv[:, qtr_t, 0, row_t] = k[rows][:, 0]
    kv[:, qtr_t, 1, row_t] = v[rows][:, 0]
    return {
        "q": np.ascontiguousarray(q[rows]),
        "kv_cache": kv,
    }


def kernel(t, q, k, v, k_cache, v_cache) -> np.ndarray:
    global LAST_RESULTS
    t_start = min(int(t) % WINDOW, BLOCK - SEQ)

    q = np.asarray(q, dtype=np.float32)
    k = np.asarray(k, dtype=np.float32)
    v = np.asarray(v, dtype=np.float32)
    k_cache = np.asarray(k_cache, dtype=np.float32)
    v_cache = np.asarray(v_cache, dtype=np.float32)

    nc = _get_nc()
    in_maps = [
        make_core_inputs(t_start, q, k, v, k_cache, v_cache, i)
        for i in range(N_CORES)
    ]

    trace = bool(int(os.environ.get("BASS_KERNEL_TRACE", "0")))
    res = run_bass_kernel_spmd(nc, in_maps, core_ids=list(range(N_CORES)), trace=trace)
    LAST_RESULTS = res
    out = np.concatenate([res.results[i]["out"] for i in range(N_CORES)], axis=0)
    # device layout is [S, SEQ, H, D]; the reference returns [S, H, SEQ, D]
    return np.ascontiguousarray(out.swapaxes(1, 2))


# revision 12
# speedup vs baseline: 1.0310x; 1.0310x over previous
"""Trainium2 Bass kernel for nn_Attention_69544110457499 (sparse_attention).

Computes, per sample n and head h (no softmax, seq=1):
    k_cache[n, t] = k[n];  v_cache[n, t] = v[n]      (t = 777 % 4096)
    out[n, h]    = (q[n,h] @ K[n,:,h,:].T) @ V[n,:,h,:]

Key ideas:
  * Data-parallel over the sample axis S=64 -> 8 samples per NeuronCore,
    fully local, zero collectives.
  * Associativity: (q @ K^T) @ V == q @ (K^T @ V). K^T V contracts over the
    cache-row axis b, which is the *natural* partition layout of both caches
    ([b, h*d] tiles straight from DRAM) -- no transposes of the 805 MB of
    cache data, and the kernel is purely HBM-bandwidth bound.
  * K and V are interleaved host-side into one kv_cache input (one DMA per
    SBUF tile), and the row-t cache write is applied during that repack, so
    the device graph has no patch traffic at all.  Only `out` is returned by
    the reference, so the updated cache never needs to reach DRAM.
  * This walrus only allows ONE sync-wait per instruction.  The structure
    keeps every instruction at <=1 wait: eight cache tiles per sample cycle
    through eight pool slots so each slot always reuses the same HWDGE DMA
    lane (same-lane FIFO ordering needs no wait), tiny "toucher" matmuls
    absorb fresh-tile DMA waits before the real accumulation matmuls (which
    carry a PSUM-slot PE self-wait), and q/out DMAs ride the separate SWDGE
    lanes.
"""

import os
import sys

sys.path.insert(0, "/opt/trn_rl_repo")

from contextlib import ExitStack

import numpy as np

import concourse.bass as bass
import concourse.mybir as mybir
import concourse.tile as tile
from concourse import bacc
from concourse.bass_utils import run_bass_kernel_spmd

N_CORES = 8
S, SEQ, H, D = 64, 1, 12, 64
BLOCK = 2048
WINDOW = 4096
NS = S // N_CORES  # samples per core
HD = H * D  # 768
P = 128  # partitions / chunk rows
CHUNKS = BLOCK // P  # 16
NQTR = 8  # cache slices per sample (DMA granules)
CPQ = CHUNKS // NQTR  # chunks per slice
QROWS = CPQ * P  # cache rows per slice
QFREE = CPQ * HD  # per-section free dim of one slice tile
NPAIR = H // 2  # head pairs

F32 = mybir.dt.float32

# Filled by kernel(); test.py reads it.
LAST_RESULTS = None


def _build_nc() -> bass.Bass:
    """Build the per-core Bass graph (t handled host-side)."""
    nc = bacc.Bacc()

    q_ext = nc.declare_dram_parameter("q", [NS, SEQ, H, D], F32, isOutput=False)
    # caches interleaved per slice: kv_cache[n, qtr, 0]=k_cache rows,
    # kv_cache[n, qtr, 1]=v_cache rows (row t already patched host-side)
    kvc_ext = nc.declare_dram_parameter(
        "kv_cache", [NS, NQTR, P, 2, CPQ, HD], F32, isOutput=False
    )
    out_ext = nc.declare_dram_parameter("out", [NS, SEQ, H, D], F32, isOutput=True)

    with tile.TileContext(nc) as tc, ExitStack() as ctx:
        cache_pool = ctx.enter_context(tc.tile_pool(name="cache", bufs=NQTR))
        ktv_pool = ctx.enter_context(tc.tile_pool(name="ktv", bufs=12))
        small_pool = ctx.enter_context(tc.tile_pool(name="small", bufs=1))
        outsb_pool = ctx.enter_context(tc.tile_pool(name="outsb", bufs=NS))
        acc_pool = ctx.enter_context(tc.tile_pool(name="acc", bufs=6, space="PSUM"))
        outp_pool = ctx.enter_context(tc.tile_pool(name="outp", bufs=2, space="PSUM"))

        # ---- q preparation (once) -------------------------------------
        # qsb: [96, 64] = q laid out (n h) x d, one contiguous DMA (SWDGE).
        qsb = small_pool.tile([NS * H, D], F32)
        nc.gpsimd.dma_start(
            out=qsb[:, :], in_=q_ext[:].rearrange("n s h d -> (n s h) d")
        )

        # qT: [64, 96] = d x (n h), via six 32x32 DVE block transposes.
        qT = small_pool.tile([D, NS * H], F32)
        for bi in range((NS * H) // 32):
            for bj in range(D // 32):
                nc.vector.transpose(
                    qT[32 * bj : 32 * (bj + 1), 32 * bi : 32 * (bi + 1)],
                    qsb[32 * bi : 32 * (bi + 1), 32 * bj : 32 * (bj + 1)],
                )

        # qx: zero-padded block-diagonal stationary for stage 2.
        # For (n, hp): columns [base, base+12); col 2hp rows 0:64 = q[n,2hp,:],
        # col 2hp+1 rows 64:128 = q[n,2hp+1,:]; everything else zero.
        qx = small_pool.tile([P, NS * NPAIR * H], F32)
        nc.vector.memset(qx[:, :], 0.0)
        for n in range(NS):
            for hp in range(NPAIR):
                base = (n * NPAIR + hp) * H
                nc.vector.tensor_copy(
                    qx[0:64, base + 2 * hp : base + 2 * hp + 1],
                    qT[0:64, n * H + 2 * hp : n * H + 2 * hp + 1],
                )
                nc.vector.tensor_copy(
                    qx[64:128, base + 2 * hp + 1 : base + 2 * hp + 2],
                    qT[0:64, n * H + 2 * hp + 1 : n * H + 2 * hp + 2],
                )

        # ---- main loop over samples -----------------------------------
        for n in range(NS):
            # Load the slice tiles of this sample's K+V cache segments.
            # Tile layout: [:, 0:QFREE] = K chunks, [:, QFREE:2*QFREE] = V.
            qtiles = []
            for qtr in range(NQTR):
                kv = cache_pool.tile(
                    [P, 2 * QFREE], F32, tag="kv", name=f"kv_{n}_{qtr}"
                )
                nc.sync.dma_start(
                    out=kv[:, :],
                    in_=kvc_ext[:][n, qtr].rearrange("p s c f -> p (s c f)"),
                )
                qtiles.append(kv)

            outp = outp_pool.tile([H, D], F32, tag="outp", name=f"outp_{n}")

            # One tiny matmul per fresh tile so the PE observes each tile's
            # DMA semaphore here; the real accumulation matmuls then carry
            # only their PSUM-slot PE self-wait (walrus allows one wait per
            # Matmult). Scribbles on outp[0,0], which stage 2 overwrites
            # (start=True clears the bank).
            for qtr in range(NQTR):
                nc.tensor.matmul(
                    outp[0:1, 0:1],
                    qtiles[qtr][0:1, 0:1],
                    qtiles[qtr][0:1, 0:1],
                    start=True,
                    stop=True,
                )

            # Two head-groups of 3 pairs each so stage-2 PSUM drains of one
            # group overlap stage-1 matmuls of the other (keeps PE warm and
            # fits 6+2 PSUM banks).
            for g in range(2):
                accs = [
                    acc_pool.tile([P, P], F32, tag="acc", name=f"acc_{n}_{g}_{j}")
                    for j in range(3)
                ]
                for qtr in range(NQTR):
                    kv = qtiles[qtr]
                    for c in range(CPQ):
                        cidx = qtr * CPQ + c
                        for i, hp in enumerate(range(3 * g, 3 * g + 3)):
                            koff = c * HD + hp * P
                            voff = QFREE + c * HD + hp * P
                            nc.tensor.matmul(
                                accs[i][:, :],
                                kv[:, koff : koff + P],
                                kv[:, voff : voff + P],
                                start=(cidx == 0),
                                stop=(cidx == CHUNKS - 1),
                            )
                # Stage 2: extract per-head diag blocks of K^T V, then the
                # tiny block-diagonal matmul q @ (K^T V) accumulating into
                # outp[12, 64].
                for i, hp in enumerate(range(3 * g, 3 * g + 3)):
                    ktv = ktv_pool.tile([P, D], F32, tag="ktv", name=f"ktv_{n}_{hp}")
                    nc.vector.tensor_copy(ktv[0:64, :], accs[i][0:64, 0:64])
                    nc.vector.tensor_copy(ktv[64:128, :], accs[i][64:128, 64:128])
                    base = (n * NPAIR + hp) * H
                    nc.tensor.matmul(
                        outp[:, :],
                        qx[:, base : base + H],
                        ktv[:, :],
                        start=(hp == 0),
                        stop=(hp == NPAIR - 1),
                    )

            osb = outsb_pool.tile([H, D], F32, tag="osb", name=f"osb_{n}")
            nc.vector.tensor_copy(osb[:, :], outp[:, :])
            nc.gpsimd.dma_start(
                out=out_ext[:][n].rearrange("s h d -> (s h) d"), in_=osb[:, :]
            )

    nc.compile()
    return nc


_NC_CACHE: dict[int, bass.Bass] = {}


def _get_nc() -> bass.Bass:
    if 0 not in _NC_CACHE:
        _NC_CACHE[0] = _build_nc()
    return _NC_CACHE[0]


def make_core_inputs(t_start, q, k, v, k_cache, v_cache, core: int):
    """Host-side shard + interleave (+ row-t cache write) for one core."""
    rows = slice(core * NS, (core + 1) * NS)

    # [NS, NQTR, P, 2, CPQ, HD]: per-partition-contiguous tile images so the
    # device DMA is a plain [128, 2*QFREE] contiguous transfer.
    kv = np.empty((NS, NQTR, P, 2, CPQ, HD), dtype=np.float32)
    k6 = k_cache[rows].reshape(NS, NQTR, CPQ, P, HD).transpose(0, 1, 3, 2, 4)
    v6 = v_cache[rows].reshape(NS, NQTR, CPQ, P, HD).transpose(0, 1, 3, 2, 4)
    kv[:, :, :, 0] = k6
    kv[:, :, :, 1] = v6
    # the KV-cache write at row t (seq=1)
    qtr_t, r = divmod(t_start, QROWS)
    c_t, p_t = divmod(r, P)
    kv[:, qtr_t, p_t, 0, c_t] = k[rows][:, 0].reshape(NS, HD)
    kv[:, qtr_t, p_t, 1, c_t] = v[rows][:, 0].reshape(NS, HD)
    return {
        "q": np.ascontiguousarray(q[rows]),
        "kv_cache": kv,
    }


def kernel(t, q, k, v, k_cache, v_cache) -> np.ndarray:
    global LAST_RESULTS
    t_start = min(int(t) % WINDOW, BLOCK - SEQ)

    q = np.asarray(q, dtype=np.float32)
    k = np.asarray(k, dtype=np.float32)
    v = np.asarray(v, dtype=np.float32)
    k_cache = np.asarray(k_cache, dtype=np.float32)
    v_cache = np.asarray(v_cache, dtype=np.float32)

    nc = _get_nc()
    in_maps = [
        make_core_inputs(t_start, q, k, v, k_cache, v_cache, i)
        for i in range(N_CORES)
    ]

    trace = bool(int(os.environ.get("BASS_KERNEL_TRACE", "0")))
    res = run_bass_kernel_spmd(nc, in_maps, core_ids=list(range(N_CORES)), trace=trace)
    LAST_RESULTS = res
    out = np.concatenate([res.results[i]["out"] for i in range(N_CORES)], axis=0)
    # device layout is [S, SEQ, H, D]; the reference returns [S, H, SEQ, D]
    return np.ascontiguousarray(out.swapaxes(1, 2))
